# revision 1
# baseline (speedup 1.0000x reference)
"""Causal self-attention (B=2, S=2048, E=1024, H=16) on 8 Trainium2 cores.

Sharding: core c in 0..7 handles batch b = c//4 and the 4 heads
[4*(c%4), 4*(c%4)+4).  The host pre-transposes x[b] and pre-slices the
QKV weights column-wise / Wo row-wise per core; each core computes its
heads' attention plus its partial output projection, and the host sums
the 4 partials per batch.

Device kernel (per core, everything resident in SBUF, matmul inputs in
fp16 with fp32 PSUM accumulation):
  xT [1024,2048] -> QT,KT [d,s] and V [s,d] projections, emitted as
  per-q-block "waves" interleaved into the attention stream.
  S^T tiles = matmul(lhsT=KT_blk, rhs=QT_blk): k on partitions, q on
  the free dim; head pairs target PE row groups 0-63/64-127
  back-to-back so their K=64 matmuls overlap in the PE array.
  exp on ScalarE (1/sqrt(D) folded into the activation scale); causal
  masking = never computing strictly-below-diagonal column ranges plus
  one 128x128 triangular mask multiply per diagonal block.
  P^T @ V with V augmented by a ones column (softmax denominator falls
  out of the same accumulation); normalize with reciprocal +
  PE-broadcast of l; Y = O @ Wo streamed out per q-block so output DMA
  overlaps remaining attention work.  Projection/Y chains are split
  into ~4-matmul units and woven between attention kb-steps to keep PE
  fed while ACT (exp) paces the softmax.
"""

import numpy as np
from contextlib import ExitStack

B, S, E, H, D = 2, 2048, 1024, 16, 64
N_CORES = 8
CPB = 4              # cores per batch
HL = H // CPB        # heads per core = 4
DL = HL * D          # local head dims = 256
P = 128              # partitions
EC = E // P          # 8 e-chunks
SB = S // P          # 16 s/k blocks
NQB = S // 512       # 4 q blocks of 512
MT = DL // P         # 2 row-tiles of QT/KT/OT (2 heads each)

_CACHE = {}
_EXHAUSTED = object()


def _emit(ctx, tc, xT, wq, wk, wv, wo, mask, y, loop_n=0):
    import concourse.bass as bass  # noqa: F401
    from concourse import mybir

    nc = tc.nc
    f32 = mybir.dt.float32
    f16 = mybir.dt.float16
    Exp = mybir.ActivationFunctionType.Exp

    res = ctx.enter_context(tc.tile_pool(name="res", bufs=1))
    xt_sb = res.tile([P, EC, S], f16, tag="xt")
    wq_sb = res.tile([P, EC, DL], f16, tag="wq")
    wk_sb = res.tile([P, EC, DL], f16, tag="wk")
    wv_sb = res.tile([P, EC, DL], f16, tag="wv")
    wo_sb = res.tile([P, MT, E], f16, tag="wo")
    qt_sb = res.tile([P, MT, S], f16, tag="qt")
    kt_sb = res.tile([P, MT, S], f16, tag="kt")
    vt_sb = res.tile([P, SB, HL, D + 1], f16, tag="vt")
    ot_sb = res.tile([P, MT, S], f16, tag="ot")
    mask_sb = res.tile([P, P], f16, tag="mask")
    ones_sb = res.tile([1, D], f16, tag="ones")

    mm_ps = ctx.enter_context(tc.tile_pool(name="mm", bufs=2, space="PSUM"))
    s_ps = ctx.enter_context(tc.tile_pool(name="sps", bufs=2, space="PSUM"))
    o_ps = ctx.enter_context(tc.tile_pool(name="ops", bufs=2, space="PSUM"))

    e_pool = ctx.enter_context(tc.tile_pool(name="ep", bufs=4))
    y_pool = ctx.enter_context(tc.tile_pool(name="yp", bufs=6))
    l_pool = ctx.enter_context(tc.tile_pool(name="lp", bufs=3))

    def _full_body():
        dma = nc.sync

        # ---- loads (interleaved so the first projection wave starts early) ----
        dma.dma_start(out=mask_sb[:], in_=mask[:])
        for ec in range(EC):
            dma.dma_start(out=xt_sb[:, ec, :], in_=xT[ec * P:(ec + 1) * P, :])
            dma.dma_start(out=wq_sb[:, ec, :], in_=wq[ec * P:(ec + 1) * P, :])
            dma.dma_start(out=wk_sb[:, ec, :], in_=wk[ec * P:(ec + 1) * P, :])
        for ec in range(EC):
            dma.dma_start(out=wv_sb[:, ec, :], in_=wv[ec * P:(ec + 1) * P, :])
        for dc in range(MT):
            dma.dma_start(out=wo_sb[:, dc, :], in_=wo[dc * P:(dc + 1) * P, :])
        nc.vector.memset(ones_sb[:], 1.0)
        nc.vector.memset(vt_sb[:, :, :, D:D + 1], 1.0)

        def wave_units(nb, parts=("qt", "kt", "v")):
            # QT/KT [:, :, nb-window] = (w chunk)^T @ xT ; V[4nb..4nb+3].
            # Generator yielding ~4-matmul units so fill stays fine-grained.
            srcs = []
            if "qt" in parts:
                srcs.append((wq_sb, qt_sb))
            if "kt" in parts:
                srcs.append((wk_sb, kt_sb))
            for mt in range(MT):
                for w_sb, t_sb in srcs:
                    ps = mm_ps.tile([P, 512], f32, tag="mm")
                    for ec in range(EC):
                        nc.tensor.matmul(
                            ps[:],
                            w_sb[:, ec, mt * P:(mt + 1) * P],
                            xt_sb[:, ec, nb * 512:(nb + 1) * 512],
                            start=(ec == 0), stop=(ec == EC - 1))
                        if ec == 3:
                            yield
                    nc.vector.tensor_copy(
                        t_sb[:, mt, nb * 512:(nb + 1) * 512], ps[:])
                    yield
            if "v" not in parts:
                return
            for sb in range(4 * nb, 4 * nb + 4):
                ps = mm_ps.tile([P, 512], f32, tag="mm")
                for ec in range(EC):
                    nc.tensor.matmul(
                        ps[:, 0:DL],
                        xt_sb[:, ec, sb * P:(sb + 1) * P],
                        wv_sb[:, ec, :],
                        start=(ec == 0), stop=(ec == EC - 1))
                    if ec == 3:
                        yield
                nc.vector.tensor_copy(
                    vt_sb[:, sb, :, 0:D],
                    ps[:, 0:DL].rearrange("p (h d) -> p h d", h=HL))
                yield

        def out_proj_units(qb):
            # Y[sb, :] = O[sb, :] @ wo for this q-block's 4 s-blocks
            for sb in range(4 * qb, 4 * qb + 4):
                for eb in range(E // 512):
                    yp = mm_ps.tile([P, 512], f32, tag="mm")
                    for dc in range(MT):
                        nc.tensor.matmul(
                            yp[:],
                            ot_sb[:, dc, sb * P:(sb + 1) * P],
                            wo_sb[:, dc, eb * 512:(eb + 1) * 512],
                            start=(dc == 0), stop=(dc == MT - 1))
                    yt = y_pool.tile([P, 512], f32, tag="y")
                    nc.vector.tensor_copy(yt[:], yp[:])
                    dma.dma_start(
                        out=y[sb * P:(sb + 1) * P, eb * 512:(eb + 1) * 512],
                        in_=yt[:])
                    yield

        def attention_block(qb, fill_units, n_fill, fill_frac=1.0):
            # ACT-paced; fill units (next wave / prev Y chains, ~4 matmuls
            # each) are emitted between the S pair and the PV pair of each
            # kb-step, so PE chews fill while ACT runs exp.  Heads go in
            # pairs: the pair's two S^T matmuls target PE row groups 0-63 /
            # 64-127 back-to-back, overlapping in the array.
            nkb = 4 * (qb + 1)     # causal: k blocks 0 .. nkb-1
            scale = float(1.0 / np.sqrt(D))
            nsteps = MT * nkb
            fill_steps = max(1, int(nsteps * fill_frac))
            done = 0

            def run_fill(step):
                nonlocal done
                want = min(n_fill, ((step + 1) * n_fill) // fill_steps)
                while done < want:
                    if next(fill_units, _EXHAUSTED) is _EXHAUSTED:
                        done = n_fill
                        break
                    done += 1

            step = 0
            for mt in range(MT):   # head pair (2*mt, 2*mt+1)
                op0 = o_ps.tile([P, 512], f32, tag="o")
                op1 = o_ps.tile([P, 512], f32, tag="o")
                ops = [op0, op1]
                for kb in range(nkb):
                    t = kb - 4 * qb
                    v0 = P * t if t > 0 else 0   # masked prefix of this window
                    sp = s_ps.tile([P, 1024], f32, tag="s")
                    for half in range(2):
                        dr = half * D
                        nc.tensor.matmul(
                            sp[:, half * 512 + v0:(half + 1) * 512],
                            kt_sb[dr:dr + D, mt, kb * P:(kb + 1) * P],
                            qt_sb[dr:dr + D, mt, qb * 512 + v0:(qb + 1) * 512],
                            start=True, stop=True)
                    et = e_pool.tile([P, 1024], f16, tag="e")
                    nc.scalar.activation(out=et[:, v0:], in_=sp[:, v0:],
                                         func=Exp, scale=scale)
                    if t >= 0:  # diagonal block: mask strictly-future keys
                        for half in range(2):
                            w0 = half * 512 + v0
                            nc.vector.tensor_mul(
                                et[:, w0:w0 + P], et[:, w0:w0 + P], mask_sb[:])
                    run_fill(step)   # PE fill while ACT computes this exp
                    step += 1
                    for half in range(2):
                        nc.tensor.matmul(
                            ops[half][0:D + 1, v0:],
                            vt_sb[:, kb, 2 * mt + half, :],
                            et[:, half * 512 + v0:(half + 1) * 512],
                            start=(kb == 0), stop=(kb == nkb - 1))
                # normalize: O^T[d, q] /= l[q]  (l = ones-column row of op)
                for half in range(2):
                    op = ops[half]
                    dr = half * D
                    lcp = l_pool.tile([1, 512], f16, tag="l")
                    nc.vector.tensor_copy(lcp[:], op[D:D + 1, :])
                    bc = mm_ps.tile([P, 512], f32, tag="mm")
                    nc.tensor.matmul(bc[0:D, :], ones_sb[:], lcp[:],
                                     start=True, stop=True)
                    rec = l_pool.tile([D, 512], f32, tag="rec")
                    nc.vector.reciprocal(rec[:], bc[0:D, :])
                    nc.vector.tensor_mul(
                        ot_sb[dr:dr + D, mt, qb * 512:(qb + 1) * 512],
                        op[0:D, :], rec[:])
            # drain leftover fill
            while next(fill_units, _EXHAUSTED) is not _EXHAUSTED:
                pass

        # wave(0) is DMA-paced (nothing else to run): interleave all four
        # qt/kt chains per e-chunk -- two accumulators from the mm pool,
        # two borrowed from the (still idle) o pool -- so each arriving
        # xt chunk feeds 4 matmuls and the chains complete right after
        # the last chunk lands.  V chains follow (chunks then resident).
        w0ps = []
        for mt in range(MT):
            pq = mm_ps.tile([P, 512], f32, tag="mm")
            pk = o_ps.tile([P, 512], f32, tag="o")
            w0ps.append((mt, wq_sb, qt_sb, pq))
            w0ps.append((mt, wk_sb, kt_sb, pk))
        for ec in range(EC):
            for mt, w_sb, t_sb, pchain in w0ps:
                nc.tensor.matmul(
                    pchain[:],
                    w_sb[:, ec, mt * P:(mt + 1) * P],
                    xt_sb[:, ec, 0:512],
                    start=(ec == 0), stop=(ec == EC - 1))
        for mt, w_sb, t_sb, pchain in w0ps:
            nc.vector.tensor_copy(t_sb[:, mt, 0:512], pchain[:])
        for sb in range(4):
            ps = mm_ps.tile([P, 512], f32, tag="mm")
            for ec in range(EC):
                nc.tensor.matmul(
                    ps[:, 0:DL],
                    xt_sb[:, ec, sb * P:(sb + 1) * P],
                    wv_sb[:, ec, :],
                    start=(ec == 0), stop=(ec == EC - 1))
            nc.vector.tensor_copy(
                vt_sb[:, sb, :, 0:D],
                ps[:, 0:DL].rearrange("p (h d) -> p h d", h=HL))
        # Fill plan: attention(qb) gets wave(qb+1) + Y(qb-1).  wave(3) is
        # split: its qt chains (needed at attn(3) step 0) stay in attn(2)'s
        # fill; its kt + V chains (only needed from kb=12) move into
        # attn(3)'s fill, front-loaded to land before kb=12 -- this drains
        # PE work from the PE-stuffed attn(2) into attn(3)'s ACT-paced
        # slack.  Unit counts: qt/kt chain = 2 units, V chain = 2 units.
        for qb in range(NQB):
            gens = []
            n_fill = 0
            frac = 1.0
            if qb + 1 < NQB - 1:
                gens.append(wave_units(qb + 1))
                n_fill += 16
            elif qb + 1 == NQB - 1:      # qb == 2: wave(3)'s qt + kt parts
                gens.append(wave_units(qb + 1, parts=("qt", "kt")))
                n_fill += 8
            else:                        # qb == 3: wave(3)'s V chains only
                gens.append(wave_units(qb, parts=("v",)))
                n_fill += 8
                frac = 0.55              # land before the kb=12 diagonal
            if qb > 0:
                gens.append(out_proj_units(qb - 1))
                n_fill += 8
            def _chain(gs=tuple(gens)):
                for g in gs:
                    yield from g
            attention_block(qb, _chain(), max(n_fill, 1), fill_frac=frac)
        for _ in out_proj_units(NQB - 1):
            pass

    if loop_n:
        # bench-only path: hint all engines so the back-edge prefetches
        # the body's IRAM blocks (body >256 instructions per engine)
        hints = (mybir.EngineType.PE, mybir.EngineType.Activation,
                 mybir.EngineType.DVE, mybir.EngineType.SP,
                 mybir.EngineType.Pool)
        with tc.For_i(0, loop_n, 1, hint_engines=hints):
            _full_body()
    else:
        _full_body()


def _get_program(loop_n=0):
    key = ("nc", loop_n)
    if key in _CACHE:
        return _CACHE[key]
    import concourse.tile as tile
    from concourse import bacc, mybir

    f32 = mybir.dt.float32
    f16 = mybir.dt.float16
    nc = bacc.Bacc("TRN2", target_bir_lowering=False, debug=False,
                   enable_asserts=False)
    xT = nc.dram_tensor("xT", [E, S], f16, kind="ExternalInput").ap()
    wq = nc.dram_tensor("wq", [E, DL], f16, kind="ExternalInput").ap()
    wk = nc.dram_tensor("wk", [E, DL], f16, kind="ExternalInput").ap()
    wv = nc.dram_tensor("wv", [E, DL], f16, kind="ExternalInput").ap()
    wo = nc.dram_tensor("wo", [DL, E], f16, kind="ExternalInput").ap()
    mask = nc.dram_tensor("mask", [P, P], f16, kind="ExternalInput").ap()
    y = nc.dram_tensor("y", [S, E], f32, kind="ExternalOutput").ap()
    with tile.TileContext(nc) as tc:
        with ExitStack() as ctx:
            _emit(ctx, tc, xT, wq, wk, wv, wo, mask, y, loop_n=loop_n)
    nc.compile()
    _CACHE[key] = nc
    return nc


def _make_in_maps(x, Wq, Wk, Wv, Wo):
    x = np.asarray(x, dtype=np.float32)
    Wq = np.asarray(Wq, dtype=np.float32)
    Wk = np.asarray(Wk, dtype=np.float32)
    Wv = np.asarray(Wv, dtype=np.float32)
    Wo = np.asarray(Wo, dtype=np.float32)
    mask = np.triu(np.ones((P, P), dtype=np.float16))
    in_maps = []
    for c in range(N_CORES):
        b, hg = divmod(c, CPB)
        hs = slice(hg * HL, (hg + 1) * HL)
        in_maps.append({
            "xT": np.ascontiguousarray(x[b].T).astype(np.float16),
            "wq": np.ascontiguousarray(Wq.reshape(E, H, D)[:, hs, :].reshape(E, DL)).astype(np.float16),
            "wk": np.ascontiguousarray(Wk.reshape(E, H, D)[:, hs, :].reshape(E, DL)).astype(np.float16),
            "wv": np.ascontiguousarray(Wv.reshape(E, H, D)[:, hs, :].reshape(E, DL)).astype(np.float16),
            "wo": np.ascontiguousarray(Wo.reshape(H, D, E)[hs, :, :].reshape(DL, E)).astype(np.float16),
            "mask": mask,
        })
    return in_maps


def run(x, Wq, Wk, Wv, Wo, trace=False):
    from concourse.bass_utils import run_bass_kernel_spmd

    nc = _get_program()
    in_maps = _make_in_maps(x, Wq, Wk, Wv, Wo)
    br = run_bass_kernel_spmd(nc, in_maps, list(range(N_CORES)), trace=trace)
    out = np.zeros((B, S, E), dtype=np.float32)
    for c in range(N_CORES):
        out[c // CPB] += br.results[c]["y"]
    return out, br


def kernel(x, Wq, Wk, Wv, Wo):
    out, _ = run(x, Wq, Wk, Wv, Wo, trace=False)
    return out



# revision 25
# speedup vs baseline: 1.5693x; 1.5693x over previous
"""Causal self-attention (B=2, S=2048, E=1024, H=16) on 8 Trainium2 cores.

Sharding: core c in 0..7 handles batch b = c//4 and the 4 heads
[4*(c%4), 4*(c%4)+4).  The host pre-transposes x[b] and pre-slices the
QKV weights column-wise / Wo row-wise per core; each core computes its
heads' attention plus its partial output projection, and the host sums
the 4 partials per batch.

Device kernel (per core, everything resident in SBUF, matmul inputs in
fp16 with fp32 PSUM accumulation):
  xT [1024,2048] -> QT,KT [d,s] and V [s,d] projections, emitted as
  per-q-block "waves" interleaved into the attention stream.
  S^T tiles = matmul(lhsT=KT_blk, rhs=QT_blk): k on partitions, q on
  the free dim.  exp on ScalarE (1/sqrt(D) folded into the activation
  scale); causal masking = never computing strictly-below-diagonal
  column ranges plus one 128x128 triangular mask multiply per diagonal
  block.  P^T V is computed q-major: per 128-q slice, matmul(lhsT=
  et[:, qslice], rhs=V_aug[k, 65]) accumulates O[q, d]+l in PSUM with
  only 65 moving columns per k-block (vs 512 the other way round); the
  softmax denominator l is the ones-column of V_aug and lands as a
  per-partition column, so normalization is a [128,4] reciprocal plus
  per-partition-scaled copies (no PE broadcast).  O[q,d] tiles are
  PE-transposed (identity matmul) back to OT[d,q] for the Y = O @ Wo
  projection, streamed out per q-block as fp16 so output DMA overlaps
  remaining attention work.  Projection/Y chains are split into
  ~4-matmul units and woven between attention kb-steps to keep PE fed
  while ACT (exp) paces the softmax.  Input DMAs are split so the
  first projection matmul starts ~3us earlier (per-chunk xT column
  halves, interleaved chunked weight loads).
"""

import numpy as np
from contextlib import ExitStack

B, S, E, H, D = 2, 2048, 1024, 16, 64
N_CORES = 8
CPB = 4              # cores per batch
HL = H // CPB        # heads per core = 4
DL = HL * D          # local head dims = 256
P = 128              # partitions
EC = E // P          # 8 e-chunks
SB = S // P          # 16 s/k blocks
NQB = S // 512       # 4 q blocks of 512
MT = DL // P         # 2 row-tiles of QT/KT/OT (2 heads each)

_CACHE = {}
_EXHAUSTED = object()


def _chain_gens(*gens):
    for g in gens:
        yield from g


def _emit(ctx, tc, xT, wqk, wv, wo, consts, y, loop_n=0, debug_outs=None):
    import concourse.bass as bass  # noqa: F401
    from concourse import mybir

    nc = tc.nc
    f32 = mybir.dt.float32
    f16 = mybir.dt.float16
    Exp = mybir.ActivationFunctionType.Exp

    res = ctx.enter_context(tc.tile_pool(name="res", bufs=1))
    xt_sb = res.tile([P, EC, S], f16, tag="xt")
    wqk_sb = res.tile([P, EC, 2, DL], f16, tag="wqk")
    wv_sb = res.tile([P, EC, DL], f16, tag="wv")
    wo_sb = res.tile([P, MT, E], f16, tag="wo")
    qt_sb = res.tile([P, MT, S], f16, tag="qt")
    kt_sb = res.tile([P, MT, S], f16, tag="kt")
    vt_sb = res.tile([P, SB, HL, D + 1], f16, tag="vt")
    ot_sb = res.tile([P, MT, S], f16, tag="ot")
    consts_sb = res.tile([P, 2, P], f16, tag="consts")
    mask_sb = consts_sb[:, 0, :]
    ident_sb = consts_sb[:, 1, :]

    mm_ps = ctx.enter_context(tc.tile_pool(name="mm", bufs=2, space="PSUM"))
    s_ps = ctx.enter_context(tc.tile_pool(name="sps", bufs=2, space="PSUM"))
    o_ps = ctx.enter_context(tc.tile_pool(name="ops", bufs=2, space="PSUM"))

    e_pool = ctx.enter_context(tc.tile_pool(name="ep", bufs=20))
    y_pool = ctx.enter_context(tc.tile_pool(name="yp", bufs=4))
    l_pool = ctx.enter_context(tc.tile_pool(name="lp", bufs=3))
    ob_pool = ctx.enter_context(tc.tile_pool(name="ob", bufs=4))

    def _full_body():
        dma = nc.sync

        # ---- loads: fine-grained so the first projection wave starts early
        # and wave-0 is never DMA-starved.  Wave-0 touches only xT columns
        # 0:512 (q-window 0 + V blocks 0..3), so those halves go first,
        # interleaved with the per-chunk q/k weight slices they pair with.
        for ec in range(EC):
            dma.dma_start(out=xt_sb[:, ec, 0:512],
                          in_=xT[ec * P:(ec + 1) * P, 0:512])
            dma.dma_start(out=wqk_sb[:, ec, :, :],
                          in_=wqk[ec * P:(ec + 1) * P, :].rearrange(
                              "p (t d) -> p t d", t=2))
            if ec == 3:
                dma.dma_start(out=wv_sb[:, 0:4, :], in_=wv[0:512, :].rearrange(
                    "(c p) d -> p c d", p=P))
        dma.dma_start(out=wv_sb[:, 4:8, :], in_=wv[512:1024, :].rearrange(
            "(c p) d -> p c d", p=P))
        dma.dma_start(out=consts_sb[:], in_=consts[:].rearrange(
            "p (t q) -> p t q", t=2))
        for ec in range(EC):
            dma.dma_start(out=xt_sb[:, ec, 512:S],
                          in_=xT[ec * P:(ec + 1) * P, 512:S])
        for dc in range(MT):
            dma.dma_start(out=wo_sb[:, dc, :], in_=wo[dc * P:(dc + 1) * P, :])
        nc.vector.memset(vt_sb[:, :, :, D:D + 1], 1.0)

        def qk_units(nb, parts=("qt", "kt")):
            # QT/KT [:, :, nb-window] = (w chunk)^T @ xT, as ~4-matmul units.
            srcs = []
            if "qt" in parts:
                srcs.append((0, qt_sb))
            if "kt" in parts:
                srcs.append((1, kt_sb))
            for mt in range(MT):
                for wi, t_sb in srcs:
                    ps = mm_ps.tile([P, 512], f32, tag="mm")
                    for ec in range(EC):
                        nc.tensor.matmul(
                            ps[:],
                            wqk_sb[:, ec, wi, mt * P:(mt + 1) * P],
                            xt_sb[:, ec, nb * 512:(nb + 1) * 512],
                            start=(ec == 0), stop=(ec == EC - 1))
                        if ec in (1, 3, 5):
                            yield
                    nc.vector.tensor_copy(
                        t_sb[:, mt, nb * 512:(nb + 1) * 512], ps[:])
                    yield

        def v_units(sb0, sb1):
            # V[sb0..sb1) = xT_blk^T @ wv, as ~4-matmul units.
            for sb in range(sb0, sb1):
                ps = mm_ps.tile([P, 512], f32, tag="mm")
                for ec in range(EC):
                    nc.tensor.matmul(
                        ps[:, 0:DL],
                        xt_sb[:, ec, sb * P:(sb + 1) * P],
                        wv_sb[:, ec, :],
                        start=(ec == 0), stop=(ec == EC - 1))
                    if ec == 3:
                        yield
                nc.vector.tensor_copy(
                    vt_sb[:, sb, :, 0:D],
                    ps[:, 0:DL].rearrange("p (h d) -> p h d", h=HL))
                yield

        def out_proj_units(qb):
            # Y[sb, :] = O[sb, :] @ wo for this q-block's 4 s-blocks; each
            # 512-wide half is copied fp16 and DMA'd immediately so the
            # final copy->DMA tail stays short.
            for sb in range(4 * qb, 4 * qb + 4):
                yt = y_pool.tile([P, E], f16, tag="y")
                for eb in range(E // 512):
                    yp = mm_ps.tile([P, 512], f32, tag="mm")
                    for dc in range(MT):
                        nc.tensor.matmul(
                            yp[:],
                            ot_sb[:, dc, sb * P:(sb + 1) * P],
                            wo_sb[:, dc, eb * 512:(eb + 1) * 512],
                            start=(dc == 0), stop=(dc == MT - 1))
                    nc.vector.tensor_copy(
                        yt[:, eb * 512:(eb + 1) * 512], yp[:])
                    dma.dma_start(
                        out=y[sb * P:(sb + 1) * P, eb * 512:(eb + 1) * 512],
                        in_=yt[:, eb * 512:(eb + 1) * 512])
                    yield

        def pv_norm_units(qb, mt, qs, ets):
            # P^T V for one (head-pair, q-slice): two contiguous
            # accumulation chains (one per head, each alone in its PSUM
            # bank -- a bank supports only ONE open accumulation group at
            # a time), then per-partition normalize by the ones-column l
            # (reciprocal + scaled copies on DVE; ACT would inflate the
            # counting-semaphore thresholds every exp-wait uses) and a PE
            # transpose (identity matmul) back to OT[d,q].
            last = 4 * qb + qs
            ohs = []
            for half in range(2):
                oh = o_ps.tile([P, D + 1], f32, tag="o")
                for kb in range(last + 1):
                    nc.tensor.matmul(
                        oh[:],
                        ets[kb][:, half * 512 + qs * P:
                                half * 512 + (qs + 1) * P],
                        vt_sb[:, kb, 2 * mt + half, :],
                        start=(kb == 0), stop=(kb == last))
                ohs.append(oh)
                yield
            ob = ob_pool.tile([P, P], f16, tag="ob")
            for half in range(2):
                rec = l_pool.tile([P, 1], f32, tag="rec")
                nc.vector.reciprocal(rec[:], ohs[half][:, D:D + 1])
                nc.vector.tensor_scalar_mul(
                    ob[:, half * D:(half + 1) * D],
                    ohs[half][:, 0:D], rec[:])
            tr = mm_ps.tile([P, P], f16, tag="mm")
            nc.tensor.transpose(tr[:], ob[:], ident_sb)
            q0 = qb * 512 + qs * P
            nc.vector.tensor_copy(ot_sb[:, mt, q0:q0 + P], tr[:])
            yield

        def attention_block(qb, fills):
            # ACT-paced; fill units (wave / Y chains, ~2-4 matmuls each) are
            # emitted around the PV batch of each kb-step, so PE chews fill
            # while ACT runs exp.  `fills` is a list of [gen, total, done,
            # deadline-substep] streams, paced linearly toward each
            # deadline; leftovers are returned for the next block.
            nkb = 4 * (qb + 1)     # causal: k blocks 0 .. nkb-1
            scale = float(1.0 / np.sqrt(D))
            nsteps = MT * nkb
            fill_steps = 2 * nsteps
            for f in fills:
                if f[3] is None:
                    f[3] = fill_steps

            def run_fill(substep):
                for f in fills:
                    gen, total, done, dl = f
                    if done >= total:
                        continue
                    want = min(total,
                               -((-total * min(substep + 1, dl)) // dl))
                    while f[2] < want:
                        if next(gen, _EXHAUSTED) is _EXHAUSTED:
                            f[2] = total
                            break
                        f[2] += 1

            step = 0
            for mt in range(MT):   # head pair (2*mt, 2*mt+1)
                ets = []           # this head-pair's exp tiles, kept live
                                   # until their last PV chain reads them
                for kb in range(nkb):
                    t = kb - 4 * qb
                    v0 = P * t if t > 0 else 0   # masked prefix of window
                    sp = s_ps.tile([P, 1024], f32, tag="s")
                    for half in range(2):
                        dr = half * D
                        nc.tensor.matmul(
                            sp[:, half * 512 + v0:(half + 1) * 512],
                            kt_sb[dr:dr + D, mt, kb * P:(kb + 1) * P],
                            qt_sb[dr:dr + D, mt, qb * 512 + v0:(qb + 1) * 512],
                            start=True, stop=True)
                    et = e_pool.tile([P, 1024], f16, tag="e")
                    ets.append(et)
                    nc.scalar.activation(out=et[:, v0:], in_=sp[:, v0:],
                                         func=Exp, scale=scale)
                    if t >= 0:  # diagonal block: mask strictly-future keys
                        for half in range(2):
                            w0 = half * 512 + v0
                            nc.vector.tensor_mul(
                                et[:, w0:w0 + P], et[:, w0:w0 + P], mask_sb)
                        # all et for q-slice qs=t now exist: its PV chains
                        # become fill, run asap (frees o banks quickly)
                        fills.insert(
                            0, [pv_norm_units(qb, mt, t, ets), 3, 0, 1])
                    run_fill(2 * step)  # PE fill while ACT runs this exp
                    run_fill(2 * step + 1)
                    step += 1
            left = [f for f in fills if f[2] < f[1]]
            for f in left:      # re-spread non-asap leftovers next block
                if f[3] != 1:
                    f[3] = None
            return left

        # wave(0): the four q/k window-0 chains interleaved per e-chunk
        # (two accumulators from the mm pool, two borrowed from the still
        # idle o pool), then V(0..5) -- paced behind the xT column-half
        # and weight-chunk DMAs issued first.
        w0ps = []
        for mt in range(MT):
            pq = mm_ps.tile([P, 512], f32, tag="mm")
            pk = o_ps.tile([P, 512], f32, tag="o")
            w0ps.append((mt, 0, qt_sb, pq))
            w0ps.append((mt, 1, kt_sb, pk))
        for ec in range(EC):
            for mt, wi, t_sb, pchain in w0ps:
                nc.tensor.matmul(
                    pchain[:],
                    wqk_sb[:, ec, wi, mt * P:(mt + 1) * P],
                    xt_sb[:, ec, 0:512],
                    start=(ec == 0), stop=(ec == EC - 1))
        for mt, wi, t_sb, pchain in w0ps:
            nc.vector.tensor_copy(t_sb[:, mt, 0:512], pchain[:])
        for _ in v_units(0, 6):
            pass
        # Fill plan: every attention block is topped up to just above its
        # ACT (exp) slack so no block ends PE-dry; kt/V windows carry
        # substep deadlines = just before the S/PV that first reads them.
        plan = [
            [[qk_units(1, ("qt",)), 8, 0, None],
             [v_units(6, 8), 4, 0, None]],
            [[qk_units(1, ("kt",)), 8, 0, 6],
             [qk_units(2, ("qt",)), 8, 0, None],
             [v_units(8, 12), 8, 0, None]],
            [[qk_units(2, ("kt",)), 8, 0, 14],
             [qk_units(3, ("qt",)), 8, 0, None],
             [out_proj_units(0), 8, 0, None]],
            [[qk_units(3, ("kt",)), 8, 0, 22],
             [v_units(12, 16), 8, 0, 24],
             [out_proj_units(1), 8, 0, None],
             [out_proj_units(2), 8, 0, None]],
        ]
        carry = []
        for qb in range(NQB):
            carry = attention_block(qb, carry + plan[qb])
            if debug_outs and "ot_mid" in debug_outs and qb == 1:
                dma.dma_start(out=debug_outs["ot_mid"],
                              in_=ot_sb.rearrange("p a b -> p (a b)"))
        for f in carry:
            while next(f[0], _EXHAUSTED) is not _EXHAUSTED:
                pass
        for _ in out_proj_units(NQB - 1):
            pass
        if debug_outs:
            dma.dma_start(out=debug_outs["qt"],
                          in_=qt_sb.rearrange("p a b -> p (a b)"))
            dma.dma_start(out=debug_outs["kt"],
                          in_=kt_sb.rearrange("p a b -> p (a b)"))
            dma.dma_start(out=debug_outs["vt"],
                          in_=vt_sb.rearrange("p a b c -> p (a b c)"))
            dma.dma_start(out=debug_outs["ot"],
                          in_=ot_sb.rearrange("p a b -> p (a b)"))

    if loop_n:
        # bench-only path: hint all engines so the back-edge prefetches
        # the body's IRAM blocks (body >256 instructions per engine)
        hints = (mybir.EngineType.PE, mybir.EngineType.Activation,
                 mybir.EngineType.DVE, mybir.EngineType.SP,
                 mybir.EngineType.Pool)
        with tc.For_i(0, loop_n, 1, hint_engines=hints):
            _full_body()
    else:
        _full_body()


def _get_program(loop_n=0):
    key = ("nc", loop_n)
    if key in _CACHE:
        return _CACHE[key]
    import concourse.tile as tile
    from concourse import bacc, mybir

    f16 = mybir.dt.float16
    nc = bacc.Bacc("TRN2", target_bir_lowering=False, debug=False,
                   enable_asserts=False)
    xT = nc.dram_tensor("xT", [E, S], f16, kind="ExternalInput").ap()
    wqk = nc.dram_tensor("wqk", [E, 2 * DL], f16, kind="ExternalInput").ap()
    wv = nc.dram_tensor("wv", [E, DL], f16, kind="ExternalInput").ap()
    wo = nc.dram_tensor("wo", [DL, E], f16, kind="ExternalInput").ap()
    consts = nc.dram_tensor("consts", [P, 2 * P], f16,
                            kind="ExternalInput").ap()
    y = nc.dram_tensor("y", [S, E], f16, kind="ExternalOutput").ap()
    with tile.TileContext(nc) as tc:
        with ExitStack() as ctx:
            _emit(ctx, tc, xT, wqk, wv, wo, consts, y, loop_n=loop_n)
    nc.compile()
    _CACHE[key] = nc
    return nc


def _make_in_maps(x, Wq, Wk, Wv, Wo):
    x = np.asarray(x, dtype=np.float32)
    Wq = np.asarray(Wq, dtype=np.float32)
    Wk = np.asarray(Wk, dtype=np.float32)
    Wv = np.asarray(Wv, dtype=np.float32)
    Wo = np.asarray(Wo, dtype=np.float32)
    consts = np.concatenate(
        [np.triu(np.ones((P, P), dtype=np.float16)),
         np.eye(P, dtype=np.float16)], axis=1)
    in_maps = []
    for c in range(N_CORES):
        b, hg = divmod(c, CPB)
        hs = slice(hg * HL, (hg + 1) * HL)
        wq_l = Wq.reshape(E, H, D)[:, hs, :].reshape(E, DL)
        wk_l = Wk.reshape(E, H, D)[:, hs, :].reshape(E, DL)
        in_maps.append({
            "xT": np.ascontiguousarray(x[b].T).astype(np.float16),
            "wqk": np.ascontiguousarray(
                np.concatenate([wq_l, wk_l], axis=1)).astype(np.float16),
            "wv": np.ascontiguousarray(
                Wv.reshape(E, H, D)[:, hs, :].reshape(E, DL)).astype(
                    np.float16),
            "wo": np.ascontiguousarray(
                Wo.reshape(H, D, E)[hs, :, :].reshape(DL, E)).astype(
                    np.float16),
            "consts": consts,
        })
    return in_maps


def run(x, Wq, Wk, Wv, Wo, trace=False):
    from concourse.bass_utils import run_bass_kernel_spmd

    nc = _get_program()
    in_maps = _make_in_maps(x, Wq, Wk, Wv, Wo)
    br = run_bass_kernel_spmd(nc, in_maps, list(range(N_CORES)), trace=trace)
    out = np.zeros((B, S, E), dtype=np.float32)
    for c in range(N_CORES):
        out[c // CPB] += br.results[c]["y"].astype(np.float32)
    return out, br


def kernel(x, Wq, Wk, Wv, Wo):
    out, _ = run(x, Wq, Wk, Wv, Wo, trace=False)
    return out


# revision 30
# speedup vs baseline: 1.6362x; 1.0427x over previous
"""Causal self-attention (B=2, S=2048, E=1024, H=16) on 8 Trainium2 cores.

Sharding: core c in 0..7 handles batch b = c//4 and the 4 heads
[4*(c%4), 4*(c%4)+4).  The host pre-transposes x[b] and pre-slices the
QKV weights column-wise / Wo row-wise per core; each core computes its
heads' attention plus its partial output projection, and the host sums
the 4 partials per batch.

Device kernel (per core, everything resident in SBUF, matmul inputs in
fp16 with fp32 PSUM accumulation):
  xT [1024,2048] -> QT,KT [d,s] and V [s,d] projections, emitted as
  per-q-block "waves" interleaved into the attention stream.
  S^T tiles = matmul(lhsT=KT_blk, rhs=QT_blk): k on partitions, q on
  the free dim.  exp on ScalarE (1/sqrt(D) folded into the activation
  scale); causal masking = never computing strictly-below-diagonal
  column ranges plus one 128x128 triangular mask multiply per diagonal
  block.  P^T V is computed q-major: per 128-q slice, matmul(lhsT=
  et[:, qslice], rhs=V_aug[k, 65]) accumulates O[q, d]+l in PSUM with
  only 65 moving columns per k-block (vs 512 the other way round); the
  softmax denominator l is the ones-column of V_aug and lands as a
  per-partition column, so normalization is a [128,4] reciprocal plus
  per-partition-scaled copies (no PE broadcast).  O[q,d] tiles are
  PE-transposed (identity matmul) back to OT[d,q] for the Y = O @ Wo
  projection, streamed out per q-block as fp16 so output DMA overlaps
  remaining attention work.  Projection/Y chains are split into
  ~4-matmul units and woven between attention kb-steps to keep PE fed
  while ACT (exp) paces the softmax.  Input DMAs are split so the
  first projection matmul starts ~3us earlier (per-chunk xT column
  halves, interleaved chunked weight loads).
"""

import numpy as np
from contextlib import ExitStack

B, S, E, H, D = 2, 2048, 1024, 16, 64
N_CORES = 8
CPB = 4              # cores per batch
HL = H // CPB        # heads per core = 4
DL = HL * D          # local head dims = 256
P = 128              # partitions
EC = E // P          # 8 e-chunks
SB = S // P          # 16 s/k blocks
NQB = S // 512       # 4 q blocks of 512
MT = DL // P         # 2 row-tiles of QT/KT/OT (2 heads each)

_CACHE = {}
_EXHAUSTED = object()


def _chain_gens(*gens):
    for g in gens:
        yield from g


def _emit(ctx, tc, xT, wqk, wv, wo, consts, y, loop_n=0, debug_outs=None):
    import concourse.bass as bass  # noqa: F401
    from concourse import mybir

    nc = tc.nc
    f32 = mybir.dt.float32
    f16 = mybir.dt.float16
    Exp = mybir.ActivationFunctionType.Exp

    res = ctx.enter_context(tc.tile_pool(name="res", bufs=1))
    xt_sb = res.tile([P, EC, S], f16, tag="xt")
    wqk_sb = res.tile([P, EC, 2, DL], f16, tag="wqk")
    wv_sb = res.tile([P, EC, DL], f16, tag="wv")
    wo_sb = res.tile([P, MT, E], f16, tag="wo")
    qt_sb = res.tile([P, MT, S], f16, tag="qt")
    kt_sb = res.tile([P, MT, S], f16, tag="kt")
    vt_sb = res.tile([P, SB, HL, D + 1], f16, tag="vt")
    ot_sb = res.tile([P, MT, S], f16, tag="ot")
    consts_sb = res.tile([P, 2, P], f16, tag="consts")
    mask_sb = consts_sb[:, 0, :]
    ident_sb = consts_sb[:, 1, :]

    mm_ps = ctx.enter_context(tc.tile_pool(name="mm", bufs=2, space="PSUM"))
    s_ps = ctx.enter_context(tc.tile_pool(name="sps", bufs=2, space="PSUM"))
    o_ps = ctx.enter_context(tc.tile_pool(name="ops", bufs=2, space="PSUM"))

    e_pool = ctx.enter_context(tc.tile_pool(name="ep", bufs=20))
    y_pool = ctx.enter_context(tc.tile_pool(name="yp", bufs=4))
    l_pool = ctx.enter_context(tc.tile_pool(name="lp", bufs=3))
    ob_pool = ctx.enter_context(tc.tile_pool(name="ob", bufs=4))

    def _full_body():
        dma = nc.sync

        # ---- loads: fine-grained so the first projection wave starts early
        # and wave-0 is never DMA-starved.  Wave-0 touches only xT columns
        # 0:512 (q-window 0 + V blocks 0..3), so those halves go first,
        # interleaved with the per-chunk q/k weight slices they pair with.
        for ec in range(EC):
            dma.dma_start(out=xt_sb[:, ec, 0:512],
                          in_=xT[ec * P:(ec + 1) * P, 0:512])
            dma.dma_start(out=wqk_sb[:, ec, :, :],
                          in_=wqk[ec * P:(ec + 1) * P, :].rearrange(
                              "p (t d) -> p t d", t=2))
            if ec == 3:
                dma.dma_start(out=wv_sb[:, 0:4, :], in_=wv[0:512, :].rearrange(
                    "(c p) d -> p c d", p=P))
        dma.dma_start(out=wv_sb[:, 4:8, :], in_=wv[512:1024, :].rearrange(
            "(c p) d -> p c d", p=P))
        dma.dma_start(out=consts_sb[:], in_=consts[:].rearrange(
            "p (t q) -> p t q", t=2))
        for ec in range(EC):
            dma.dma_start(out=xt_sb[:, ec, 512:S],
                          in_=xT[ec * P:(ec + 1) * P, 512:S])
        for dc in range(MT):
            dma.dma_start(out=wo_sb[:, dc, :], in_=wo[dc * P:(dc + 1) * P, :])
        nc.vector.memset(vt_sb[:, :, :, D:D + 1], 1.0)

        def qk_units(nb, parts=("qt", "kt")):
            # QT/KT [:, :, nb-window] = (w chunk)^T @ xT, as ~4-matmul units.
            srcs = []
            if "qt" in parts:
                srcs.append((0, qt_sb))
            if "kt" in parts:
                srcs.append((1, kt_sb))
            for mt in range(MT):
                for wi, t_sb in srcs:
                    ps = mm_ps.tile([P, 512], f32, tag="mm")
                    for ec in range(EC):
                        nc.tensor.matmul(
                            ps[:],
                            wqk_sb[:, ec, wi, mt * P:(mt + 1) * P],
                            xt_sb[:, ec, nb * 512:(nb + 1) * 512],
                            start=(ec == 0), stop=(ec == EC - 1))
                        if ec in (1, 3, 5):
                            yield
                    nc.vector.tensor_copy(
                        t_sb[:, mt, nb * 512:(nb + 1) * 512], ps[:])
                    yield

        def v_units(sb0, sb1):
            # V[sb0..sb1) = xT_blk^T @ wv, as ~4-matmul units.
            for sb in range(sb0, sb1):
                ps = mm_ps.tile([P, 512], f32, tag="mm")
                for ec in range(EC):
                    nc.tensor.matmul(
                        ps[:, 0:DL],
                        xt_sb[:, ec, sb * P:(sb + 1) * P],
                        wv_sb[:, ec, :],
                        start=(ec == 0), stop=(ec == EC - 1))
                    if ec == 3:
                        yield
                nc.vector.tensor_copy(
                    vt_sb[:, sb, :, 0:D],
                    ps[:, 0:DL].rearrange("p (h d) -> p h d", h=HL))
                yield

        def out_proj_units(qb, act_copies=False):
            # Y[sb, :] = O[sb, :] @ wo for this q-block's 4 s-blocks; each
            # 512-wide half is copied fp16 and DMA'd immediately so the
            # final copy->DMA tail stays short.  act_copies splits the
            # PSUM->SBUF copies across DVE and the (by then idle) ACT.
            for sb in range(4 * qb, 4 * qb + 4):
                yt = y_pool.tile([P, E], f16, tag="y")
                for eb in range(E // 512):
                    yp = mm_ps.tile([P, 512], f32, tag="mm")
                    for dc in range(MT):
                        nc.tensor.matmul(
                            yp[:],
                            ot_sb[:, dc, sb * P:(sb + 1) * P],
                            wo_sb[:, dc, eb * 512:(eb + 1) * 512],
                            start=(dc == 0), stop=(dc == MT - 1))
                    dst = yt[:, eb * 512:(eb + 1) * 512]
                    if act_copies and eb == 1:
                        nc.scalar.copy(dst, yp[:])
                    else:
                        nc.vector.tensor_copy(dst, yp[:])
                    dma.dma_start(
                        out=y[sb * P:(sb + 1) * P, eb * 512:(eb + 1) * 512],
                        in_=dst)
                    yield

        def pv_norm_units(qb, mt, qs, ets):
            # P^T V for one (head-pair, q-slice): two contiguous
            # accumulation chains (one per head, each alone in its PSUM
            # bank -- a bank supports only ONE open accumulation group at
            # a time), then per-partition normalize by the ones-column l
            # (reciprocal + scaled copies on DVE; ACT would inflate the
            # counting-semaphore thresholds every exp-wait uses) and a PE
            # transpose (identity matmul) back to OT[d,q].
            last = 4 * qb + qs
            ohs = []
            for half in range(2):
                oh = o_ps.tile([P, D + 1], f32, tag="o")
                for kb in range(last + 1):
                    nc.tensor.matmul(
                        oh[:],
                        ets[kb][:, half * 512 + qs * P:
                                half * 512 + (qs + 1) * P],
                        vt_sb[:, kb, 2 * mt + half, :],
                        start=(kb == 0), stop=(kb == last))
                ohs.append(oh)
                yield
            ob = ob_pool.tile([P, P], f16, tag="ob")
            for half in range(2):
                rec = l_pool.tile([P, 1], f32, tag="rec")
                nc.vector.reciprocal(rec[:], ohs[half][:, D:D + 1])
                nc.vector.tensor_scalar_mul(
                    ob[:, half * D:(half + 1) * D],
                    ohs[half][:, 0:D], rec[:])
            tr = mm_ps.tile([P, P], f16, tag="mm")
            nc.tensor.transpose(tr[:], ob[:], ident_sb)
            q0 = qb * 512 + qs * P
            nc.vector.tensor_copy(ot_sb[:, mt, q0:q0 + P], tr[:])
            yield

        def attention_block(qb, fills):
            # ACT-paced; fill units (wave / Y chains, ~2-4 matmuls each) are
            # emitted around the PV batch of each kb-step, so PE chews fill
            # while ACT runs exp.  `fills` is a list of [gen, total, done,
            # deadline-substep] streams, paced linearly toward each
            # deadline; leftovers are returned for the next block.
            nkb = 4 * (qb + 1)     # causal: k blocks 0 .. nkb-1
            scale = float(1.0 / np.sqrt(D))
            nsteps = MT * nkb
            fill_steps = 2 * nsteps
            for f in fills:
                if f[3] is None:
                    f[3] = fill_steps
                if len(f) < 5:
                    f.append(0)

            def run_fill(substep):
                for f in fills:
                    gen, total, done, dl, st = f
                    if done >= total or substep < st:
                        continue
                    span = max(1, dl - st)
                    want = min(total, -((-total * min(substep + 1 - st,
                                                      span)) // span))
                    while f[2] < want:
                        if next(gen, _EXHAUSTED) is _EXHAUSTED:
                            f[2] = total
                            break
                        f[2] += 1

            step = 0
            for mt in range(MT):   # head pair (2*mt, 2*mt+1)
                ets = []           # this head-pair's exp tiles, kept live
                                   # until their last PV chain reads them
                for kb in range(nkb):
                    t = kb - 4 * qb
                    v0 = P * t if t > 0 else 0   # masked prefix of window
                    sp = s_ps.tile([P, 1024], f32, tag="s")
                    for half in range(2):
                        dr = half * D
                        nc.tensor.matmul(
                            sp[:, half * 512 + v0:(half + 1) * 512],
                            kt_sb[dr:dr + D, mt, kb * P:(kb + 1) * P],
                            qt_sb[dr:dr + D, mt, qb * 512 + v0:(qb + 1) * 512],
                            start=True, stop=True)
                    et = e_pool.tile([P, 1024], f16, tag="e")
                    ets.append(et)
                    nc.scalar.activation(out=et[:, v0:], in_=sp[:, v0:],
                                         func=Exp, scale=scale)
                    if t >= 0:  # diagonal block: mask strictly-future keys
                        for half in range(2):
                            w0 = half * 512 + v0
                            nc.vector.tensor_mul(
                                et[:, w0:w0 + P], et[:, w0:w0 + P], mask_sb)
                        # all et for q-slice qs=t now exist: its PV chains
                        # become fill, run asap (frees o banks quickly)
                        fills.insert(
                            0, [pv_norm_units(qb, mt, t, ets), 3, 0, 1, 0])
                    run_fill(2 * step)  # PE fill while ACT runs this exp
                    run_fill(2 * step + 1)
                    step += 1
            left = [f for f in fills if f[2] < f[1]]
            for f in left:      # re-spread non-asap leftovers next block
                if f[3] != 1:
                    f[3] = None
                    f[4] = 0
            return left

        # wave(0): the four q/k window-0 chains interleaved per e-chunk
        # (two accumulators from the mm pool, two borrowed from the still
        # idle o pool), then V(0..5) -- paced behind the xT column-half
        # and weight-chunk DMAs issued first.
        w0ps = []
        for mt in range(MT):
            pq = mm_ps.tile([P, 512], f32, tag="mm")
            pk = o_ps.tile([P, 512], f32, tag="o")
            w0ps.append((mt, 0, qt_sb, pq))
            w0ps.append((mt, 1, kt_sb, pk))
        for ec in range(EC):
            for mt, wi, t_sb, pchain in w0ps:
                nc.tensor.matmul(
                    pchain[:],
                    wqk_sb[:, ec, wi, mt * P:(mt + 1) * P],
                    xt_sb[:, ec, 0:512],
                    start=(ec == 0), stop=(ec == EC - 1))
        for mt, wi, t_sb, pchain in w0ps:
            nc.vector.tensor_copy(t_sb[:, mt, 0:512], pchain[:])
        for _ in v_units(0, 6):
            pass
        # Fill plan: every attention block is topped up to just above its
        # ACT (exp) slack so no block ends PE-dry; kt/V windows carry
        # substep deadlines = just before the S/PV that first reads them.
        plan = [
            [[qk_units(1, ("qt",)), 8, 0, None, 0],
             [v_units(6, 8), 4, 0, None, 0]],
            [[qk_units(1, ("kt",)), 8, 0, 6, 0],
             [qk_units(2, ("qt",)), 8, 0, None, 0],
             [v_units(8, 12), 8, 0, None, 0]],
            [[qk_units(2, ("kt",)), 8, 0, 14, 0],
             [qk_units(3, ("qt",)), 8, 0, None, 0],
             [out_proj_units(0), 8, 0, None, 16]],
            [[qk_units(3, ("kt",)), 8, 0, 22, 0],
             [v_units(12, 16), 8, 0, 24, 0],
             [out_proj_units(1), 8, 0, None, 24],
             [out_proj_units(2), 8, 0, None, 40],
             [out_proj_units(3, act_copies=True), 8, 0, None, 56]],
        ]
        carry = []
        for qb in range(NQB):
            carry = attention_block(qb, carry + plan[qb])
        for f in carry:
            while next(f[0], _EXHAUSTED) is not _EXHAUSTED:
                pass
        if debug_outs:
            dma.dma_start(out=debug_outs["qt"],
                          in_=qt_sb.rearrange("p a b -> p (a b)"))
            dma.dma_start(out=debug_outs["kt"],
                          in_=kt_sb.rearrange("p a b -> p (a b)"))
            dma.dma_start(out=debug_outs["vt"],
                          in_=vt_sb.rearrange("p a b c -> p (a b c)"))
            dma.dma_start(out=debug_outs["ot"],
                          in_=ot_sb.rearrange("p a b -> p (a b)"))

    if loop_n:
        # bench-only path: hint all engines so the back-edge prefetches
        # the body's IRAM blocks (body >256 instructions per engine)
        hints = (mybir.EngineType.PE, mybir.EngineType.Activation,
                 mybir.EngineType.DVE, mybir.EngineType.SP,
                 mybir.EngineType.Pool)
        with tc.For_i(0, loop_n, 1, hint_engines=hints):
            _full_body()
    else:
        _full_body()


def _get_program(loop_n=0):
    key = ("nc", loop_n)
    if key in _CACHE:
        return _CACHE[key]
    import concourse.tile as tile
    from concourse import bacc, mybir

    f16 = mybir.dt.float16
    nc = bacc.Bacc("TRN2", target_bir_lowering=False, debug=False,
                   enable_asserts=False)
    xT = nc.dram_tensor("xT", [E, S], f16, kind="ExternalInput").ap()
    wqk = nc.dram_tensor("wqk", [E, 2 * DL], f16, kind="ExternalInput").ap()
    wv = nc.dram_tensor("wv", [E, DL], f16, kind="ExternalInput").ap()
    wo = nc.dram_tensor("wo", [DL, E], f16, kind="ExternalInput").ap()
    consts = nc.dram_tensor("consts", [P, 2 * P], f16,
                            kind="ExternalInput").ap()
    y = nc.dram_tensor("y", [S, E], f16, kind="ExternalOutput").ap()
    with tile.TileContext(nc) as tc:
        with ExitStack() as ctx:
            _emit(ctx, tc, xT, wqk, wv, wo, consts, y, loop_n=loop_n)
    nc.compile()
    _CACHE[key] = nc
    return nc


def _make_in_maps(x, Wq, Wk, Wv, Wo):
    x = np.asarray(x, dtype=np.float32)
    Wq = np.asarray(Wq, dtype=np.float32)
    Wk = np.asarray(Wk, dtype=np.float32)
    Wv = np.asarray(Wv, dtype=np.float32)
    Wo = np.asarray(Wo, dtype=np.float32)
    consts = np.concatenate(
        [np.triu(np.ones((P, P), dtype=np.float16)),
         np.eye(P, dtype=np.float16)], axis=1)
    in_maps = []
    for c in range(N_CORES):
        b, hg = divmod(c, CPB)
        hs = slice(hg * HL, (hg + 1) * HL)
        wq_l = Wq.reshape(E, H, D)[:, hs, :].reshape(E, DL)
        wk_l = Wk.reshape(E, H, D)[:, hs, :].reshape(E, DL)
        in_maps.append({
            "xT": np.ascontiguousarray(x[b].T).astype(np.float16),
            "wqk": np.ascontiguousarray(
                np.concatenate([wq_l, wk_l], axis=1)).astype(np.float16),
            "wv": np.ascontiguousarray(
                Wv.reshape(E, H, D)[:, hs, :].reshape(E, DL)).astype(
                    np.float16),
            "wo": np.ascontiguousarray(
                Wo.reshape(H, D, E)[hs, :, :].reshape(DL, E)).astype(
                    np.float16),
            "consts": consts,
        })
    return in_maps


def run(x, Wq, Wk, Wv, Wo, trace=False):
    from concourse.bass_utils import run_bass_kernel_spmd

    nc = _get_program()
    in_maps = _make_in_maps(x, Wq, Wk, Wv, Wo)
    br = run_bass_kernel_spmd(nc, in_maps, list(range(N_CORES)), trace=trace)
    out = np.zeros((B, S, E), dtype=np.float32)
    for c in range(N_CORES):
        out[c // CPB] += br.results[c]["y"].astype(np.float32)
    return out, br


def kernel(x, Wq, Wk, Wv, Wo):
    out, _ = run(x, Wq, Wk, Wv, Wo, trace=False)
    return out


# revision 37
# speedup vs baseline: 1.6460x; 1.0060x over previous
"""Causal self-attention (B=2, S=2048, E=1024, H=16) on 8 Trainium2 cores.

Sharding: core c in 0..7 handles batch b = c//4 and the 4 heads
[4*(c%4), 4*(c%4)+4).  The host pre-transposes x[b] and pre-slices the
QKV weights column-wise / Wo row-wise per core; each core computes its
heads' attention plus its partial output projection, and the host sums
the 4 partials per batch.

Device kernel (per core, everything resident in SBUF, matmul inputs in
fp16 with fp32 PSUM accumulation):
  xT [1024,2048] -> QT,KT [d,s] and V [s,d] projections, emitted as
  per-q-block "waves" interleaved into the attention stream.
  S^T tiles = matmul(lhsT=KT_blk, rhs=QT_blk): k on partitions, q on
  the free dim.  exp on ScalarE (1/sqrt(D) folded into the activation
  scale); causal masking = never computing strictly-below-diagonal
  column ranges plus one 128x128 triangular mask multiply per diagonal
  block.  P^T V is computed q-major: per 128-q slice, matmul(lhsT=
  et[:, qslice], rhs=V_aug[k, 65]) accumulates O[q, d]+l in PSUM with
  only 65 moving columns per k-block (vs 512 the other way round); the
  softmax denominator l is the ones-column of V_aug and lands as a
  per-partition column, so normalization is a [128,4] reciprocal plus
  per-partition-scaled copies (no PE broadcast).  O[q,d] tiles are
  PE-transposed (identity matmul) back to OT[d,q] for the Y = O @ Wo
  projection, streamed out per q-block as fp16 so output DMA overlaps
  remaining attention work.  Projection/Y chains are split into
  ~4-matmul units and woven between attention kb-steps to keep PE fed
  while ACT (exp) paces the softmax.  Input DMAs are split so the
  first projection matmul starts ~3us earlier (per-chunk xT column
  halves, interleaved chunked weight loads).
"""

import numpy as np
from contextlib import ExitStack

B, S, E, H, D = 2, 2048, 1024, 16, 64
N_CORES = 8
CPB = 4              # cores per batch
HL = H // CPB        # heads per core = 4
DL = HL * D          # local head dims = 256
P = 128              # partitions
EC = E // P          # 8 e-chunks
SB = S // P          # 16 s/k blocks
NQB = S // 512       # 4 q blocks of 512
MT = DL // P         # 2 row-tiles of QT/KT/OT (2 heads each)

_CACHE = {}
_EXHAUSTED = object()


def _chain_gens(*gens):
    for g in gens:
        yield from g


def _emit(ctx, tc, xT, wqk, wv, wo, consts, y, loop_n=0, debug_outs=None):
    import concourse.bass as bass  # noqa: F401
    from concourse import mybir

    nc = tc.nc
    f32 = mybir.dt.float32
    f16 = mybir.dt.float16
    Exp = mybir.ActivationFunctionType.Exp

    res = ctx.enter_context(tc.tile_pool(name="res", bufs=1))
    xt_sb = res.tile([P, EC, S], f16, tag="xt")
    wqk_sb = res.tile([P, EC, 2, DL], f16, tag="wqk")
    wv_sb = res.tile([P, EC, DL], f16, tag="wv")
    wo_sb = res.tile([P, MT, E], f16, tag="wo")
    qt_sb = res.tile([P, MT, S], f16, tag="qt")
    kt_sb = res.tile([P, MT, S], f16, tag="kt")
    vt_sb = res.tile([P, SB, HL, D + 1], f16, tag="vt")
    ot_sb = res.tile([P, MT, S], f16, tag="ot")
    consts_sb = res.tile([P, 2, P], f16, tag="consts")
    mask_sb = consts_sb[:, 0, :]
    ident_sb = consts_sb[:, 1, :]

    mm_ps = ctx.enter_context(tc.tile_pool(name="mm", bufs=2, space="PSUM"))
    s_ps = ctx.enter_context(tc.tile_pool(name="sps", bufs=2, space="PSUM"))
    o_ps = ctx.enter_context(tc.tile_pool(name="ops", bufs=2, space="PSUM"))

    e_pool = ctx.enter_context(tc.tile_pool(name="ep", bufs=20))
    y_pool = ctx.enter_context(tc.tile_pool(name="yp", bufs=4))
    l_pool = ctx.enter_context(tc.tile_pool(name="lp", bufs=3))
    ob_pool = ctx.enter_context(tc.tile_pool(name="ob", bufs=4))

    def _full_body():
        dma = nc.sync

        # ---- loads: fine-grained so the first projection wave starts early
        # and wave-0 is never DMA-starved.  Wave-0 touches only xT columns
        # 0:512 (q-window 0 + V blocks 0..3), so those halves go first,
        # interleaved with the per-chunk q/k weight slices they pair with.
        for ec in range(EC):
            dma.dma_start(out=xt_sb[:, ec, 0:512],
                          in_=xT[ec * P:(ec + 1) * P, 0:512])
            dma.dma_start(out=wqk_sb[:, ec, :, :],
                          in_=wqk[ec * P:(ec + 1) * P, :].rearrange(
                              "p (t d) -> p t d", t=2))
            if ec == 3:
                dma.dma_start(out=wv_sb[:, 0:4, :], in_=wv[0:512, :].rearrange(
                    "(c p) d -> p c d", p=P))
        dma.dma_start(out=wv_sb[:, 4:8, :], in_=wv[512:1024, :].rearrange(
            "(c p) d -> p c d", p=P))
        dma.dma_start(out=consts_sb[:], in_=consts[:].rearrange(
            "p (t q) -> p t q", t=2))
        for ec in range(EC):
            dma.dma_start(out=xt_sb[:, ec, 512:S],
                          in_=xT[ec * P:(ec + 1) * P, 512:S])
        for dc in range(MT):
            dma.dma_start(out=wo_sb[:, dc, :], in_=wo[dc * P:(dc + 1) * P, :])
        nc.vector.memset(vt_sb[:, :, :, D:D + 1], 1.0)

        def qk_units(nb, parts=("qt", "kt")):
            # QT/KT [:, :, nb-window] = (w chunk)^T @ xT, as ~4-matmul units.
            srcs = []
            if "qt" in parts:
                srcs.append((0, qt_sb))
            if "kt" in parts:
                srcs.append((1, kt_sb))
            for mt in range(MT):
                for wi, t_sb in srcs:
                    ps = mm_ps.tile([P, 512], f32, tag="mm")
                    for ec in range(EC):
                        nc.tensor.matmul(
                            ps[:],
                            wqk_sb[:, ec, wi, mt * P:(mt + 1) * P],
                            xt_sb[:, ec, nb * 512:(nb + 1) * 512],
                            start=(ec == 0), stop=(ec == EC - 1))
                        if ec in (1, 3, 5):
                            yield
                    nc.vector.tensor_copy(
                        t_sb[:, mt, nb * 512:(nb + 1) * 512], ps[:])
                    yield

        def v_units(sb0, sb1):
            # V[sb0..sb1) = xT_blk^T @ wv, as ~4-matmul units.
            for sb in range(sb0, sb1):
                ps = mm_ps.tile([P, 512], f32, tag="mm")
                for ec in range(EC):
                    nc.tensor.matmul(
                        ps[:, 0:DL],
                        xt_sb[:, ec, sb * P:(sb + 1) * P],
                        wv_sb[:, ec, :],
                        start=(ec == 0), stop=(ec == EC - 1))
                    if ec == 3:
                        yield
                nc.vector.tensor_copy(
                    vt_sb[:, sb, :, 0:D],
                    ps[:, 0:DL].rearrange("p (h d) -> p h d", h=HL))
                yield

        def out_proj_units(qb, act_copies=False):
            # Y[sb, :] = O[sb, :] @ wo for this q-block's 4 s-blocks; each
            # 512-wide half is copied fp16 and DMA'd immediately so the
            # final copy->DMA tail stays short.  act_copies splits the
            # PSUM->SBUF copies across DVE and the (by then idle) ACT.
            for sb in range(4 * qb, 4 * qb + 4):
                yt = y_pool.tile([P, E], f16, tag="y")
                for eb in range(E // 512):
                    yp = mm_ps.tile([P, 512], f32, tag="mm")
                    for dc in range(MT):
                        nc.tensor.matmul(
                            yp[:],
                            ot_sb[:, dc, sb * P:(sb + 1) * P],
                            wo_sb[:, dc, eb * 512:(eb + 1) * 512],
                            start=(dc == 0), stop=(dc == MT - 1))
                    dst = yt[:, eb * 512:(eb + 1) * 512]
                    if act_copies and eb == 1:
                        nc.scalar.copy(dst, yp[:])
                    else:
                        nc.vector.tensor_copy(dst, yp[:])
                    dma.dma_start(
                        out=y[sb * P:(sb + 1) * P, eb * 512:(eb + 1) * 512],
                        in_=dst)
                    yield

        def pv_norm_units(qb, mt, qs, ets):
            # P^T V for one (head-pair, q-slice): two contiguous
            # accumulation chains (one per head, each alone in its PSUM
            # bank -- a bank supports only ONE open accumulation group at
            # a time), then per-partition normalize by the ones-column l
            # (reciprocal + scaled copies on DVE; ACT would inflate the
            # counting-semaphore thresholds every exp-wait uses) and a PE
            # transpose (identity matmul) back to OT[d,q].
            last = 4 * qb + qs
            ohs = []
            for half in range(2):
                oh = o_ps.tile([P, D + 1], f32, tag="o")
                for kb in range(last + 1):
                    nc.tensor.matmul(
                        oh[:],
                        ets[kb][:, half * 512 + qs * P:
                                half * 512 + (qs + 1) * P],
                        vt_sb[:, kb, 2 * mt + half, :],
                        start=(kb == 0), stop=(kb == last))
                ohs.append(oh)
                yield
            ob = ob_pool.tile([P, P], f16, tag="ob")
            for half in range(2):
                rec = l_pool.tile([P, 1], f32, tag="rec")
                nc.vector.reciprocal(rec[:], ohs[half][:, D:D + 1])
                nc.vector.tensor_scalar_mul(
                    ob[:, half * D:(half + 1) * D],
                    ohs[half][:, 0:D], rec[:])
            tr = mm_ps.tile([P, P], f16, tag="mm")
            nc.tensor.transpose(tr[:], ob[:], ident_sb)
            q0 = qb * 512 + qs * P
            nc.vector.tensor_copy(ot_sb[:, mt, q0:q0 + P], tr[:])
            yield

        def attention_block(qb, fills):
            # ACT-paced; fill units (wave / Y chains, ~2-4 matmuls each) are
            # emitted around the PV batch of each kb-step, so PE chews fill
            # while ACT runs exp.  `fills` is a list of [gen, total, done,
            # deadline-substep] streams, paced linearly toward each
            # deadline; leftovers are returned for the next block.
            nkb = 4 * (qb + 1)     # causal: k blocks 0 .. nkb-1
            scale = float(1.0 / np.sqrt(D))
            nsteps = MT * nkb
            fill_steps = 2 * nsteps
            for f in fills:
                if f[3] is None:
                    f[3] = fill_steps
                if len(f) < 5:
                    f.append(0)

            def run_fill(substep):
                for f in fills:
                    gen, total, done, dl, st = f
                    if done >= total or substep < st:
                        continue
                    span = max(1, dl - st)
                    want = min(total, -((-total * min(substep + 1 - st,
                                                      span)) // span))
                    while f[2] < want:
                        if next(gen, _EXHAUSTED) is _EXHAUSTED:
                            f[2] = total
                            break
                        f[2] += 1

            run_fill(0)   # pre-fill: cover the inter-block exp catch-up
            step = 0
            for mt in range(MT):   # head pair (2*mt, 2*mt+1)
                ets = []           # this head-pair's exp tiles, kept live
                                   # until their last PV chain reads them
                for kb in range(nkb):
                    t = kb - 4 * qb
                    v0 = P * t if t > 0 else 0   # masked prefix of window
                    sp = s_ps.tile([P, 1024], f32, tag="s")
                    for half in range(2):
                        dr = half * D
                        nc.tensor.matmul(
                            sp[:, half * 512 + v0:(half + 1) * 512],
                            kt_sb[dr:dr + D, mt, kb * P:(kb + 1) * P],
                            qt_sb[dr:dr + D, mt, qb * 512 + v0:(qb + 1) * 512],
                            start=True, stop=True)
                    et = e_pool.tile([P, 1024], f16, tag="e")
                    ets.append(et)
                    nc.scalar.activation(out=et[:, v0:], in_=sp[:, v0:],
                                         func=Exp, scale=scale)
                    if t >= 0:  # diagonal block: mask strictly-future keys
                        for half in range(2):
                            w0 = half * 512 + v0
                            nc.vector.tensor_mul(
                                et[:, w0:w0 + P], et[:, w0:w0 + P], mask_sb)
                        # all et for q-slice qs=t now exist: its PV chains
                        # become fill, run asap (frees o banks quickly)
                        fills.insert(
                            0, [pv_norm_units(qb, mt, t, ets), 3, 0, 1, 0])
                    run_fill(2 * step)  # PE fill while ACT runs this exp
                    run_fill(2 * step + 1)
                    step += 1
            left = [f for f in fills if f[2] < f[1]]
            for f in left:      # re-spread non-asap leftovers next block
                if f[3] != 1:
                    f[3] = None
                    f[4] = 0
            return left

        # wave(0): the four q/k window-0 chains interleaved per e-chunk
        # (two accumulators from the mm pool, two borrowed from the still
        # idle o pool), then V(0..5) -- paced behind the xT column-half
        # and weight-chunk DMAs issued first.
        w0ps = []
        for mt in range(MT):
            pq = mm_ps.tile([P, 512], f32, tag="mm")
            pk = o_ps.tile([P, 512], f32, tag="o")
            w0ps.append((mt, 0, qt_sb, pq))
            w0ps.append((mt, 1, kt_sb, pk))
        for ec in range(EC):
            for mt, wi, t_sb, pchain in w0ps:
                nc.tensor.matmul(
                    pchain[:],
                    wqk_sb[:, ec, wi, mt * P:(mt + 1) * P],
                    xt_sb[:, ec, 0:512],
                    start=(ec == 0), stop=(ec == EC - 1))
        for mt, wi, t_sb, pchain in w0ps:
            nc.vector.tensor_copy(t_sb[:, mt, 0:512], pchain[:])
        for _ in v_units(0, 6):
            pass
        # Fill plan: every attention block is topped up to just above its
        # ACT (exp) slack so no block ends PE-dry; kt/V windows carry
        # substep deadlines = just before the S/PV that first reads them.
        plan = [
            [[qk_units(1, ("qt",)), 8, 0, None, 0],
             [v_units(6, 8), 4, 0, None, 0]],
            [[qk_units(1, ("kt",)), 8, 0, 6, 0],
             [qk_units(2, ("qt",)), 8, 0, None, 0],
             [v_units(8, 12), 8, 0, None, 0]],
            [[qk_units(2, ("kt",)), 8, 0, 14, 0],
             [qk_units(3, ("qt",)), 8, 0, None, 0],
             [out_proj_units(0), 8, 0, None, 16]],
            [[qk_units(3, ("kt",)), 8, 0, 22, 0],
             [v_units(12, 16), 8, 0, 24, 0],
             [out_proj_units(1), 8, 0, None, 24],
             [out_proj_units(2), 8, 0, None, 40],
             [out_proj_units(3, act_copies=True), 8, 0, None, 56]],
        ]
        carry = []
        for qb in range(NQB):
            carry = attention_block(qb, carry + plan[qb])
        for f in carry:
            while next(f[0], _EXHAUSTED) is not _EXHAUSTED:
                pass
        if debug_outs:
            dma.dma_start(out=debug_outs["qt"],
                          in_=qt_sb.rearrange("p a b -> p (a b)"))
            dma.dma_start(out=debug_outs["kt"],
                          in_=kt_sb.rearrange("p a b -> p (a b)"))
            dma.dma_start(out=debug_outs["vt"],
                          in_=vt_sb.rearrange("p a b c -> p (a b c)"))
            dma.dma_start(out=debug_outs["ot"],
                          in_=ot_sb.rearrange("p a b -> p (a b)"))

    if loop_n:
        # bench-only path: hint all engines so the back-edge prefetches
        # the body's IRAM blocks (body >256 instructions per engine)
        hints = (mybir.EngineType.PE, mybir.EngineType.Activation,
                 mybir.EngineType.DVE, mybir.EngineType.SP,
                 mybir.EngineType.Pool)
        with tc.For_i(0, loop_n, 1, hint_engines=hints):
            _full_body()
    else:
        _full_body()


def _get_program(loop_n=0):
    key = ("nc", loop_n)
    if key in _CACHE:
        return _CACHE[key]
    import concourse.tile as tile
    from concourse import bacc, mybir

    f16 = mybir.dt.float16
    nc = bacc.Bacc("TRN2", target_bir_lowering=False, debug=False,
                   enable_asserts=False)
    xT = nc.dram_tensor("xT", [E, S], f16, kind="ExternalInput").ap()
    wqk = nc.dram_tensor("wqk", [E, 2 * DL], f16, kind="ExternalInput").ap()
    wv = nc.dram_tensor("wv", [E, DL], f16, kind="ExternalInput").ap()
    wo = nc.dram_tensor("wo", [DL, E], f16, kind="ExternalInput").ap()
    consts = nc.dram_tensor("consts", [P, 2 * P], f16,
                            kind="ExternalInput").ap()
    y = nc.dram_tensor("y", [S, E], f16, kind="ExternalOutput").ap()
    with tile.TileContext(nc) as tc:
        with ExitStack() as ctx:
            _emit(ctx, tc, xT, wqk, wv, wo, consts, y, loop_n=loop_n)
    nc.compile()
    _CACHE[key] = nc
    return nc


def _make_in_maps(x, Wq, Wk, Wv, Wo):
    x = np.asarray(x, dtype=np.float32)
    Wq = np.asarray(Wq, dtype=np.float32)
    Wk = np.asarray(Wk, dtype=np.float32)
    Wv = np.asarray(Wv, dtype=np.float32)
    Wo = np.asarray(Wo, dtype=np.float32)
    consts = np.concatenate(
        [np.triu(np.ones((P, P), dtype=np.float16)),
         np.eye(P, dtype=np.float16)], axis=1)
    in_maps = []
    for c in range(N_CORES):
        b, hg = divmod(c, CPB)
        hs = slice(hg * HL, (hg + 1) * HL)
        wq_l = Wq.reshape(E, H, D)[:, hs, :].reshape(E, DL)
        wk_l = Wk.reshape(E, H, D)[:, hs, :].reshape(E, DL)
        in_maps.append({
            "xT": np.ascontiguousarray(x[b].T).astype(np.float16),
            "wqk": np.ascontiguousarray(
                np.concatenate([wq_l, wk_l], axis=1)).astype(np.float16),
            "wv": np.ascontiguousarray(
                Wv.reshape(E, H, D)[:, hs, :].reshape(E, DL)).astype(
                    np.float16),
            "wo": np.ascontiguousarray(
                Wo.reshape(H, D, E)[hs, :, :].reshape(DL, E)).astype(
                    np.float16),
            "consts": consts,
        })
    return in_maps


def run(x, Wq, Wk, Wv, Wo, trace=False):
    from concourse.bass_utils import run_bass_kernel_spmd

    nc = _get_program()
    in_maps = _make_in_maps(x, Wq, Wk, Wv, Wo)
    br = run_bass_kernel_spmd(nc, in_maps, list(range(N_CORES)), trace=trace)
    out = np.zeros((B, S, E), dtype=np.float32)
    for c in range(N_CORES):
        out[c // CPB] += br.results[c]["y"].astype(np.float32)
    return out, br


def kernel(x, Wq, Wk, Wv, Wo):
    out, _ = run(x, Wq, Wk, Wv, Wo, trace=False)
    return out
